# revision 52
# baseline (speedup 1.0000x reference)
"""Depthwise 13x13 stride-4 conv (AntiAliasInterpolation2d) on 8 TRN2 NeuronCores.

Pure data parallel: batch 32 -> 4 images per core. Two device graphs:

1. rank-1 path (used when each channel's 13x13 kernel is an outer product
   v ⊗ h, which holds for the Gaussian anti-alias kernel): separable conv.
   Stage V contracts input rows on the TensorEngine via banded-Toeplitz
   stationaries (stride-4 vertical conv, fp32 PSUM accumulate); the DVE
   copies V to SBUF in bf16 while de-interleaving columns into 4 phases;
   stage H applies the horizontal taps as diagonal-stationary matmuls,
   one per tap, accumulating in PSUM (the stride-4 column gather becomes
   a contiguous slice in phase space).

2. general path (fallback for non-separable weights): direct 2D conv as
   52 PSUM-accumulated banded-Toeplitz matmuls per channel (13 kernel
   columns x 4 row chunks), stride-4 columns de-interleaved on the host.

Everything computes in bf16 (fp32 accumulation); output is fp32.
"""

import numpy as np
import ml_dtypes

N_CORES = 8
B, C, H, W = 32, 3, 512, 512
KS = 13          # kernel size
PAD = 6          # pad on each side
STR = 4          # stride
OH = OW = 128    # output spatial
PW = W + 2 * PAD  # 524 padded width
NPH = PW // STR   # 131 columns per phase
BPC = B // N_CORES  # images per core = 4
XW = BPC * PW     # 2096 free-dim columns per input tile

# general path epack layout
SLOT = 130
NPAIR = C * KS
EPACK_COLS = (NPAIR - 1) * SLOT + 224

_CACHE = {}


def _bacc():
    from concourse import bacc

    return bacc.Bacc(
        "TRN2", target_bir_lowering=False, debug=False, num_devices=N_CORES
    )


STCOLS = 4 * 128 + KS * 32  # per-channel stationaries: 4 Toeplitz + 13 diag32


def _build_graph_rank1():
    import concourse.tile as tile
    from concourse import mybir

    nc = _bacc()
    # input: [c, 128 rows, k(row chunk) * img * 524 padded cols] bf16
    x = nc.dram_tensor("x", [C, 128, 4 * XW], mybir.dt.bfloat16, kind="ExternalInput")
    # per-channel stationaries, partition-major: [av: 4*128 | diag32: 13*32]
    st = nc.dram_tensor("st", [C, 128, STCOLS], mybir.dt.bfloat16, kind="ExternalInput")
    out = nc.dram_tensor("out", [BPC, C, OH, OW], mybir.dt.bfloat16, kind="ExternalOutput")

    f32 = mybir.dt.float32
    bf16 = mybir.dt.bfloat16

    with tile.TileContext(nc) as tc:
        with (
            tc.tile_pool(name="stp", bufs=3) as stp,
            tc.tile_pool(name="xin", bufs=3) as xin,
            tc.tile_pool(name="vsb", bufs=2) as vsbp,
            tc.tile_pool(name="vpsA", bufs=4, space="PSUM") as vpsA,
            tc.tile_pool(name="vpsB", bufs=2, space="PSUM") as vpsB,
            tc.tile_pool(name="hps", bufs=2, space="PSUM") as hps,
            tc.tile_pool(name="ot", bufs=2) as otp,
        ):
            for c in range(C):
                stt = stp.tile([128, STCOLS], bf16)
                nc.scalar.dma_start(stt[:], st[c])
                xt = xin.tile([128, 4 * XW], bf16)
                if c == 0:
                    # split the first channel into row-chunk pieces so its
                    # first matmuls aren't gated on the whole 2 MiB transfer
                    for k in range(4):
                        nc.sync.dma_start(
                            xt[:, k * XW : (k + 1) * XW],
                            x[c, :, k * XW : (k + 1) * XW],
                        )
                else:
                    nc.sync.dma_start(xt[:], x[c])

                # ---- stage V: vertical 13-tap stride-4 conv via Toeplitz ----
                # x free layout: [k 4][img 4][524]
                vsb = vsbp.tile([128, XW], bf16)
                vB = vpsB.tile([128, BPC * 12], f32)
                vgall = vsb[:].rearrange("p (g ph u) -> p g ph u", g=BPC, ph=STR)
                vA = [
                    vpsA.tile([128, 512], f32, tag="vA", name=f"vA_{c}_{g}")
                    for g in range(BPC)
                ]
                # chunks 0-2 image-inner (gated only on their own DMA piece);
                # chunk 3 per image, so each image's cast fires early and
                # overlaps the next image's matmuls on the DVE
                for k in range(3):
                    for g in range(BPC):
                        nc.tensor.matmul(
                            vA[g][:],
                            stt[:, k * 128 : (k + 1) * 128],
                            xt[:, (k * BPC + g) * PW : (k * BPC + g) * PW + 512],
                            start=(k == 0),
                            stop=False,
                        )
                for g in range(BPC):
                    nc.tensor.matmul(
                        vA[g][:],
                        stt[:, 3 * 128 : 4 * 128],
                        xt[:, (3 * BPC + g) * PW : (3 * BPC + g) * PW + 512],
                        start=False,
                        stop=True,
                    )
                    # copy V to SBUF bf16, de-interleaving into 4 phases
                    srcA = vA[g][:].rearrange("p (u ph) -> p ph u", ph=STR)
                    nc.vector.tensor_copy(vgall[:, g, :, 0:128], srcA)
                # rightmost 12 padded columns of each image, all images at once
                xg = xt[:].rearrange("p (k g w) -> p k g w", k=4, g=BPC)
                for k in range(4):
                    nc.tensor.matmul(
                        vB[:],
                        stt[:, k * 128 : (k + 1) * 128],
                        xg[:, k, :, 512:524],
                        start=(k == 0),
                        stop=(k == 3),
                    )
                srcB = vB[:].rearrange("p (g u ph) -> p g ph u", g=BPC, ph=STR)
                nc.vector.tensor_copy(vgall[:, :, :, 128:131], srcB)

                # ---- stage H: 13 taps, diag32 stationaries col-tiled 4x ----
                hp = hps.tile([128, BPC * OW], f32)
                vg = vsb[:].rearrange("p (g w) -> p g w", g=BPC)
                for j in range(KS):
                    ph, q = j % STR, j // STR
                    off = ph * NPH + q
                    for i in range(4):
                        nc.tensor.matmul(
                            hp[32 * i : 32 * i + 32, :],
                            stt[32 * i : 32 * i + 32, 512 + j * 32 : 512 + j * 32 + 32],
                            vg[32 * i : 32 * i + 32, :, off : off + OW],
                            start=(j == 0),
                            stop=(j == KS - 1),
                            skip_group_check=True,
                            tile_position=(32 * i, 32 * i),
                        )

                o = otp.tile([128, BPC * OW], bf16)
                nc.vector.tensor_copy(o[:], hp[:])
                dst = out[:, c].rearrange("g y x -> y g x")
                nc.sync.dma_start(dst, o[:].rearrange("y (g x) -> y g x", g=BPC))
    nc.finalize()
    return nc


def _build_graph_rank1_raw():
    """Hand-scheduled raw-bacc version: ~16 semaphores, no Tile framework.

    Static buffers: all 3 channels' inputs resident in SBUF (DMAs issued
    back-to-back at t=0), double-buffered V/out staging, 6 PSUM banks
    (4 vertical accumulators + B-strip + horizontal accumulator).
    """
    import concourse.bass as bass  # noqa: F401
    from concourse import mybir
    from contextlib import ExitStack

    nc = _bacc()
    x = nc.dram_tensor("x", [C, 128, 4 * XW], mybir.dt.bfloat16, kind="ExternalInput")
    st = nc.dram_tensor("st", [C, 128, STCOLS], mybir.dt.bfloat16, kind="ExternalInput")
    out = nc.dram_tensor("out", [BPC, C, OH, OW], mybir.dt.bfloat16, kind="ExternalOutput")

    f32 = mybir.dt.float32
    bf16 = mybir.dt.bfloat16
    CW = 4 * XW  # input elems per channel

    with nc.cleanup_on_exit(), ExitStack() as es:
        xt = es.enter_context(nc.sbuf_tensor("xt", [128, 3 * CW], bf16))
        stt = es.enter_context(nc.sbuf_tensor("stt", [128, 3 * STCOLS], bf16))
        vsb = es.enter_context(nc.sbuf_tensor("vsb", [128, 2 * XW], bf16))
        ot = es.enter_context(nc.sbuf_tensor("ot", [128, 2 * 512], bf16))
        vA = es.enter_context(nc.psum_tensor("vA", [128, 4 * 512], f32))
        vB = es.enter_context(nc.psum_tensor("vB", [128, 512], f32))
        hp2 = es.enter_context(nc.psum_tensor("hp2", [128, 2 * 512], f32))

        s_x = [
            [es.enter_context(nc.semaphore(f"s_x{c}_{k}")) for k in range(4)]
            for c in range(C)
        ]
        s_x00 = [es.enter_context(nc.semaphore(f"s_x00_{g}")) for g in range(BPC)]
        s_st = [es.enter_context(nc.semaphore(f"s_st{c}")) for c in range(C)]
        s_mm = [es.enter_context(nc.semaphore(f"s_mm{c}")) for c in range(C)]
        s_vc = [es.enter_context(nc.semaphore(f"s_vc{c}")) for c in range(C)]
        s_out = [es.enter_context(nc.semaphore(f"s_out{c}")) for c in range(C)]
        s_od = [es.enter_context(nc.semaphore(f"s_od{c}")) for c in range(C)]

        with nc.Block() as block:

            @block.sync
            def _(sync):
                # one ring, consumption order: the SDMA drains these FIFO at
                # ~line rate, pacing the PE's k-waves. c0/k0 goes in two
                # halves so the first matmuls are gated on 262 KB only.
                for h in range(2):
                    sync.dma_start(
                        xt[:, 2 * h * PW : 2 * (h + 1) * PW],
                        x[0][:, 2 * h * PW : 2 * (h + 1) * PW],
                    ).then_inc(s_x00[h], 16)
                for c in range(C):
                    for k in range(4):
                        if c == 0 and k == 0:
                            continue
                        sync.dma_start(
                            xt[:, c * CW + k * XW : c * CW + (k + 1) * XW],
                            x[c][:, k * XW : (k + 1) * XW],
                        ).then_inc(s_x[c][k], 16)
                for c in range(C):
                    sync.wait_ge(s_out[c], 1)
                    dst = out[:, c].rearrange("g y x -> y g x")
                    src = ot[:, (c % 2) * 512 : (c % 2) * 512 + 512].rearrange(
                        "y (g xx) -> y g xx", g=BPC
                    )
                    sync.dma_start(dst, src).then_inc(s_od[c], 16)
                for c in range(C):
                    sync.wait_ge(s_od[c], 16)

            @block.scalar
            def _(scalar):
                for c in range(C):
                    scalar.dma_start(
                        stt[:, c * STCOLS : (c + 1) * STCOLS], st[c]
                    ).then_inc(s_st[c], 16)

            @block.tensor
            def _(tensor):
                def emit_VB(c):
                    av0 = c * STCOLS  # av columns for this channel
                    xc0 = c * CW
                    tensor.wait_ge(s_st[c], 16)
                    for k in range(4):
                        if not (c == 0 and k == 0):
                            tensor.wait_ge(s_x[c][k], 16)
                        lhsT = stt[:, av0 + k * 128 : av0 + (k + 1) * 128]
                        for g in range(BPC):
                            if c == 0 and k == 0 and g % 2 == 0:
                                tensor.wait_ge(s_x00[g // 2], 16)
                            if k == 0 and c >= 1:
                                tensor.wait_ge(s_vc[c - 1], g + 1)
                            mm = tensor.matmul(
                                vA[:, g * 512 : g * 512 + 512],
                                lhsT,
                                xt[:, xc0 + (k * BPC + g) * PW : xc0 + (k * BPC + g) * PW + 512],
                                start=(k == 0),
                                stop=(k == 3),
                                skip_group_check=True,
                            )
                            if k == 3:
                                mm.then_inc(s_mm[c], 1)
                    if c >= 1:
                        tensor.wait_ge(s_vc[c - 1], 5)
                    xg = xt[:, xc0 : xc0 + CW].rearrange(
                        "p (k g w) -> p k g w", k=4, g=BPC
                    )
                    for k in range(4):
                        mm = tensor.matmul(
                            vB[:, 0 : BPC * 12],
                            stt[:, av0 + k * 128 : av0 + (k + 1) * 128],
                            xg[:, k, :, 512:524],
                            start=(k == 0),
                            stop=(k == 3),
                            skip_group_check=True,
                        )
                    mm.then_inc(s_mm[c], 1)

                def emit_H(c, tri=False):
                    dg0 = c * STCOLS + 512  # diag32 columns
                    b0 = (c % 2) * XW
                    h0 = (c % 2) * 512
                    # groups: (img range, x range, s_vc gate). x < 125 only
                    # touches phase columns u < 128, so it can start before
                    # castB; the x >= 125 sliver needs castB (count 5).
                    if tri:
                        groups = [
                            (0, 2, 0, 125, 2),
                            (2, 4, 0, 125, 4),
                            (0, 4, 125, 128, 5),
                        ]
                    else:
                        groups = [(0, 4, 0, 128, 5)]
                    for gi, (glo, ghi, xlo, xhi, gate) in enumerate(groups):
                        tensor.wait_ge(s_vc[c], gate)
                        if c >= 2 and gi == 0:
                            tensor.wait_ge(s_out[c - 2], 1)  # hp bank WAR
                        mm = None
                        for j in range(KS):
                            ph, q = j % STR, j // STR
                            off = ph * NPH + q
                            for i in range(4):
                                rhs = vsb[
                                    32 * i : 32 * i + 32, b0 : b0 + XW
                                ].rearrange("p (g w) -> p g w", g=BPC)[
                                    :, glo:ghi, off + xlo : off + xhi
                                ]
                                dst = hp2[
                                    32 * i : 32 * i + 32, h0 : h0 + 512
                                ].rearrange("p (g xx) -> p g xx", g=BPC)[
                                    :, glo:ghi, xlo:xhi
                                ]
                                mm = tensor.matmul(
                                    dst,
                                    stt[32 * i : 32 * i + 32, dg0 + j * 32 : dg0 + j * 32 + 32],
                                    rhs,
                                    start=(j == 0),
                                    stop=(j == KS - 1),
                                    skip_group_check=True,
                                    tile_position=(32 * i, 32 * i),
                                )
                        mm.then_inc(s_mm[c], 1)

                # software pipeline: fill the cast(c) latency with V(c+1)
                emit_VB(0)
                emit_VB(1)
                emit_H(0)
                emit_VB(2)
                emit_H(1)
                emit_H(2, tri=True)

            @block.vector
            def _(vector):
                def emit_casts(c):
                    b0 = (c % 2) * XW
                    vg = vsb[:, b0 : b0 + XW].rearrange(
                        "p (g ph u) -> p g ph u", g=BPC, ph=STR
                    )
                    for g in range(BPC):
                        vector.wait_ge(s_mm[c], g + 1)
                        if c >= 2 and g == 0:
                            vector.wait_ge(s_mm[c - 2], 6)  # vsb WAR vs H(c-2)
                        srcA = vA[:, g * 512 : g * 512 + 512].rearrange(
                            "p (u ph) -> p ph u", ph=STR
                        )
                        vector.tensor_copy(vg[:, g, :, 0:128], srcA).then_inc(
                            s_vc[c], 1
                        )
                    vector.wait_ge(s_mm[c], 5)
                    srcB = vB[:, 0 : BPC * 12].rearrange(
                        "p (g u ph) -> p g ph u", g=BPC, ph=STR
                    )
                    vector.tensor_copy(vg[:, :, :, 128:131], srcB).then_inc(
                        s_vc[c], 1
                    )

                def emit_out(c, ngroups=1):
                    h0 = (c % 2) * 512
                    vector.wait_ge(s_mm[c], 5 + ngroups)
                    if c >= 2:
                        vector.wait_ge(s_od[c - 2], 16)
                    vector.tensor_copy(
                        ot[:, (c % 2) * 512 : (c % 2) * 512 + 512],
                        hp2[:, h0 : h0 + 512],
                    ).then_inc(s_out[c], 1)

                # mirror the PE pipeline: VB0 VB1 H0 VB2 H1 H2
                emit_casts(0)
                emit_casts(1)
                emit_out(0)
                emit_casts(2)
                emit_out(1)
                emit_out(2, ngroups=3)

        nc.all_engine_barrier()
    nc.finalize()
    return nc


def _build_graph_general():
    import concourse.tile as tile
    from concourse import mybir

    nc = _bacc()
    x = nc.dram_tensor("x", [C, 4, 128, XW], mybir.dt.bfloat16, kind="ExternalInput")
    ep = nc.dram_tensor("ep", [128, EPACK_COLS], mybir.dt.bfloat16, kind="ExternalInput")
    out = nc.dram_tensor("out", [BPC, C, OH, OW], mybir.dt.float32, kind="ExternalOutput")

    with tile.TileContext(nc) as tc:
        with (
            tc.tile_pool(name="const", bufs=1) as constp,
            tc.tile_pool(name="xin", bufs=4) as xin,
            tc.tile_pool(name="ps", bufs=2, space="PSUM") as psp,
            tc.tile_pool(name="ot", bufs=2) as otp,
        ):
            ept = constp.tile([128, EPACK_COLS], mybir.dt.bfloat16)
            nc.scalar.dma_start(ept[:], ep[:])
            for c in range(C):
                psum = psp.tile([128, BPC * OW], mybir.dt.float32)
                for k in range(4):
                    xt = xin.tile([128, XW], mybir.dt.bfloat16)
                    nc.sync.dma_start(xt[:], x[c, k])
                    xg = xt[:].rearrange("p (g w) -> p g w", g=BPC)
                    for j in range(KS):
                        ph, q = j % STR, j // STR
                        off = ph * NPH + q
                        rhs = xg[:, :, off : off + OW]
                        t = c * KS + j
                        lo = t * SLOT + 96 - 32 * k
                        lhsT = ept[:, lo : lo + 128]
                        nc.tensor.matmul(
                            psum[:],
                            lhsT,
                            rhs,
                            start=(k == 0 and j == 0),
                            stop=(k == 3 and j == KS - 1),
                        )
                o = otp.tile([128, BPC * OW], mybir.dt.float32)
                nc.vector.tensor_copy(o[:], psum[:])
                dst = out[:, c].rearrange("g y x -> y g x")
                nc.sync.dma_start(dst, o[:].rearrange("y (g x) -> y g x", g=BPC))
    nc.finalize()
    return nc


def _decompose(weight):
    """Per-channel SVD; return (v[c,13], h[c,13]) if rank-1, else None."""
    vs, hs = [], []
    for c in range(C):
        w = weight[c, 0].astype(np.float64)
        u, s, vt = np.linalg.svd(w)
        if s[1] > 1e-5 * s[0]:
            return None
        sc = np.sqrt(s[0])
        vs.append(u[:, 0] * sc)
        hs.append(vt[0] * sc)
    return np.stack(vs), np.stack(hs)


def _pad_shard(inp):
    """[32,3,512,512] f32 -> [core, c, 128, k*img*524] bf16 (padded cols)."""
    bf16 = ml_dtypes.bfloat16
    pad = np.zeros((B, C, H, PW), np.float32)
    pad[..., PAD : PAD + W] = inp
    arr = pad.reshape(N_CORES, BPC, C, 4, 128, PW)
    arr = arr.transpose(0, 2, 4, 3, 1, 5).reshape(N_CORES, C, 128, 4 * XW)
    return np.ascontiguousarray(arr).astype(bf16)


def _phase_shard(inp):
    """[32,3,512,512] f32 -> padded + phase-deinterleaved shards (general)."""
    bf16 = ml_dtypes.bfloat16
    pad = np.zeros((B, C, H, PW), np.float32)
    pad[..., PAD : PAD + W] = inp
    phmat = pad.reshape(B, C, H, NPH, STR).transpose(0, 1, 2, 4, 3)
    arr = phmat.reshape(N_CORES, BPC, C, 4, 128, STR, NPH)
    arr = arr.transpose(0, 2, 3, 4, 1, 5, 6).reshape(N_CORES, C, 4, 128, XW)
    return np.ascontiguousarray(arr).astype(bf16)


def _toeplitz_band(vec):
    """[13] taps -> [4, 128, 128] vertical Toeplitz chunks A[k][r, y]."""
    a = np.zeros((4, 128, 128), np.float32)
    for k in range(4):
        r = np.arange(128)[:, None] + 128 * k
        y = np.arange(128)[None, :]
        i = r - 4 * y + PAD
        m = (i >= 0) & (i < KS)
        a[k][m] = vec[i[m]]
    return a


def _prep_rank1(inp, v, h):
    bf16 = ml_dtypes.bfloat16
    arr = _pad_shard(inp)
    st = np.zeros((C, 128, STCOLS), np.float32)
    eye32 = np.eye(32, dtype=np.float32)
    for c in range(C):
        # [0, 512): vertical Toeplitz chunks, k-major
        st[c, :, : 4 * 128] = (
            _toeplitz_band(v[c]).transpose(1, 0, 2).reshape(128, 4 * 128)
        )
        # [512, 512+13*32): 32x32 diag h[c, j], replicated down partitions
        for j in range(KS):
            blk = np.tile(h[c, j] * eye32, (4, 1))  # [128, 32]
            st[c, :, 512 + j * 32 : 512 + (j + 1) * 32] = blk
    st = st.astype(bf16)
    return [{"x": arr[core], "st": st} for core in range(N_CORES)]


def _prep_general(inp, weight):
    bf16 = ml_dtypes.bfloat16
    arr = _phase_shard(inp)
    epk = np.zeros((128, EPACK_COLS), np.float32)
    r = np.arange(128)
    for c in range(C):
        for j in range(KS):
            t = c * KS + j
            for s in range(-2, 34):
                i = r - 4 * s + PAD
                m = (i >= 0) & (i < KS)
                if m.any():
                    epk[m, t * SLOT + 96 + s] = weight[c, 0, i[m], j]
    epk = epk.astype(bf16)
    return [{"x": arr[core], "ep": epk} for core in range(N_CORES)]


def _prep(inp, weight):
    """Returns (graph_key, in_maps)."""
    inp = np.asarray(inp, dtype=np.float32)
    weight = np.asarray(weight, dtype=np.float32)
    vh = _decompose(weight)
    if vh is not None:
        return "rank1", _prep_rank1(inp, *vh)
    return "general", _prep_general(inp, weight)


_BUILDERS = {
    "rank1": lambda: _build_graph_rank1_raw(),
    "rank1_tile": lambda: _build_graph_rank1(),
    "general": lambda: _build_graph_general(),
}


def _graph(key):
    if key not in _CACHE:
        _CACHE[key] = _BUILDERS[key]()
    return _CACHE[key]


def kernel(inp, weight):
    from concourse.bass_utils import run_bass_kernel_spmd

    key, in_maps = _prep(inp, weight)
    nc = _graph(key)
    res = run_bass_kernel_spmd(nc, in_maps, core_ids=list(range(N_CORES)))
    outs = [res.results[i]["out"] for i in range(N_CORES)]
    return np.concatenate(outs, axis=0).astype(np.float32)
